# revision 14
# baseline (speedup 1.0000x reference)
"""Trainium2 Bass kernel for sparse_attention nn_CWAB_34050500722860 (v6).

Model (per batch b, S=4096 tokens, D=1024, H=16 heads of 64):
  xp = x + pos_emb[:S]
  local: non-overlapping 512-token window self-attention per head
  global: keys = concat(conv4-compressed first-256-tokens (G=64), global_memory)
  out = sigmoid([local||glob] @ gate_w + gate_b) gated mix, then @ out_w + out_b

Sharding: 8 cores = 4 batches x 2 sequence halves (2048 tokens = 4 windows
per core). Every core computes its batch's 128 global tokens from a
replicated copy of the first 256 tokens; no collectives.

Structure (scores transposed st[k,q]; attention/out-proj bf16, gate fp8):
  - HOST pre-adds pos, casts to bf16, and pre-transposes: xa (token-major,
    with a 64-wide ones block interleaved per head) and xt (feature-major,
    head-pair layout) - no device-side transposes or adds for x.
  - doubled-ones PV: lhsT = [V_head(64) | ones(64)] per head, so the PV
    matmul (M=128, same N cost) yields rows 0-63 = attention numerator and
    rows 64-127 = softmax denominator ALREADY BROADCAST to 64 partitions.
    Normalization = exp(-ln(Z)) on the scalar engine straight from PSUM
    plus one fused multiply-evacuate per half on DVE. No selector matmuls,
    no Z-row copies. (DVE InstReciprocal measured ~9x its modeled cost on
    this hw - avoid; GPSIMD cannot access PSUM at all.)
  - gate matmul in fp8e4m3 DoubleRow (2 k-rows/partition, 0.5 cycles/row):
    weights host-quantized into DoubleRow layout; attention outputs copied
    bf16->fp8 on the otherwise-idle GPSIMD engine. Measured rel err
    0.0118 vs the 0.02 budget (hw == sim).
  - gate sigmoid as exp(-ln(1+exp(-x))) so every activation in the kernel
    stays inside the natural_log_exp ACT table set (ln is needed by the
    normalize; mixing in tanh would cost 1283ns table loads per switch).
    gate_b is pre-negated on the host and rides the pass-1 bias.
  - cross-window software pipelining: window w's gate/comb/out-proj units
    (pure PE work) are emitted interleaved with window w+1's ACT-throttled
    attention blocks, keeping both engines fed.
  - PSUM: scores 2x[128,2,512] + pv 2x[128,512] + gate/out 2x[128,512]
    = exactly 8 banks; gtT-transposes borrow the pv ring via bitcast.
"""

import os
from contextlib import ExitStack

import numpy as np
import ml_dtypes

import jax
import jax.numpy as jnp
from jax.sharding import Mesh, PartitionSpec
from jax.experimental.shard_map import shard_map

import bass_rust
import concourse.bass as bass
import concourse.mybir as mybir
import concourse.tile as tile
from concourse import bass2jax
from concourse.masks import make_identity
from concourse.vector_clock import ScopedClock

F32 = mybir.dt.float32
FP8 = mybir.dt.float8e4
BF16 = mybir.dt.bfloat16
BF16_NP = ml_dtypes.bfloat16
FP8_NP = ml_dtypes.float8_e4m3

N_CORES = 8
B, S, D = 4, 4096, 1024
H, HD, WIN, G = 16, 64, 512, 64
SC = S // 2          # tokens per core
NW = SC // WIN       # 4 windows per core
SCALE = 1.0 / np.sqrt(HD)
ACT_EXP = mybir.ActivationFunctionType.Exp
ACT_TANH = mybir.ActivationFunctionType.Tanh
ACT_LN = mybir.ActivationFunctionType.Ln


class SplitDrainTileContext(tile.TileContext):
    """Walrus in this container rejects a Drain carrying >2 sem waits.

    Tile's exit puts a wait for every engine/queue sem on one Drain; strip
    them and re-emit as single-wait NOPs on the sync sequencer (sequencer
    order keeps the all-engine barrier behind every wait)."""

    def _drain_and_barrier(self, tick_clock, wait_clock):
        nc = self.nc
        drain_inst = nc.sync.drain()
        wait_clock.add_sem_waits(
            drain_inst.ins, ScopedClock({None: tick_clock.global_clock})
        )
        si = drain_inst.ins.sync_info
        waits = list(si.on_wait) if si is not None and si.on_wait else []
        if len(waits) > 1:
            updates = list(si.on_update) if si is not None and si.on_update else []
            drain_inst.ins.sync_info = bass_rust.SyncInfo(
                on_wait=waits[:1], on_update=updates
            )
            for w in waits[1:]:
                nop = nc.sync.nop()
                nop.ins.sync_info = bass_rust.SyncInfo(on_wait=[w], on_update=[])

        nc.all_engine_barrier()
        assert self.sems is not None
        popped = nc._tile_sem_poison_stack.pop()
        assert popped is self._sem_poison
        nc.clear_and_free_semaphores(list(self.sems.allocated().values()))
        nc.all_engine_barrier()


def split_sync_waits(nc, limit: int = 1):
    """This walrus build accepts at most one sem wait per instruction.

    For any instruction carrying more, peel the extras onto same-engine
    NoOps inserted directly before it (engine streams preserve bb order,
    so the engine still blocks on every wait before executing it)."""
    for fn in nc.m.functions:
        for bb in fn.blocks:
            il = bb.instructions
            out, changed, k = [], False, 0
            for inst in il:
                si = inst.sync_info
                waits = list(si.on_wait) if si is not None and si.on_wait else []
                if len(waits) > limit and inst.engine != mybir.EngineType.Unassigned:
                    for w in waits[:-limit]:
                        nop = mybir.InstNoOp(
                            name=f"{inst.name}-wsplit{k}", engine=inst.engine
                        )
                        nop.sync_info = bass_rust.SyncInfo(on_wait=[w], on_update=[])
                        out.append(nop)
                        k += 1
                    inst.sync_info = bass_rust.SyncInfo(
                        on_wait=waits[-limit:],
                        on_update=list(si.on_update) if si.on_update else [],
                    )
                    changed = True
                out.append(inst)
            if changed:
                bb.instructions = out


def build_nc(nrep: int = 1):
    nc = bass.Bass("TRN2", target_bir_lowering=False, debug=False)

    xa_d = nc.declare_dram_parameter("xa", [SC, 2 * D], BF16, isOutput=False).ap()
    xt_d = nc.declare_dram_parameter("xt", [128, 8, SC], BF16, isOutput=False).ap()
    xgp_d = nc.declare_dram_parameter("xgp", [128, 4 * 8 * G], BF16, isOutput=False).ap()
    gma_d = nc.declare_dram_parameter("gma", [G, 16 * 128], BF16, isOutput=False).ap()
    gmt_d = nc.declare_dram_parameter("gmt", [128, 8 * HD], BF16, isOutput=False).ap()
    cw_d = nc.declare_dram_parameter("cw", [4 * D, D], BF16, isOutput=False).ap()
    cb_d = nc.declare_dram_parameter("cb", [1, D], F32, isOutput=False).ap()
    gw_d = nc.declare_dram_parameter("gw", [128, 16 * D], FP8, isOutput=False).ap()
    gb_d = nc.declare_dram_parameter("gb", [8, 128], F32, isOutput=False).ap()
    ow_d = nc.declare_dram_parameter("ow", [D, D], BF16, isOutput=False).ap()
    ob_d = nc.declare_dram_parameter("ob", [1, D], F32, isOutput=False).ap()
    out_d = nc.declare_dram_parameter("out", [SC, D], F32, isOutput=True).ap()

    xa_r = xa_d.rearrange("(n p) d -> n p d", p=128)     # [16,128,1024]
    out_r = out_d.rearrange("(n p) d -> n p d", p=128)

    with SplitDrainTileContext(nc) as tc, ExitStack() as ctx:
        const = ctx.enter_context(tc.tile_pool(name="const", bufs=1))
        wpool = ctx.enter_context(tc.tile_pool(name="weights", bufs=1))
        cwp = ctx.enter_context(tc.tile_pool(name="cw_stream", bufs=3))
        xwp = ctx.enter_context(tc.tile_pool(name="xw", bufs=2))
        xtp = ctx.enter_context(tc.tile_pool(name="xt", bufs=2))
        expp = ctx.enter_context(tc.tile_pool(name="expt", bufs=4))
        rbp = ctx.enter_context(tc.tile_pool(name="rb", bufs=4))
        ltp = ctx.enter_context(tc.tile_pool(name="localT", bufs=2))
        gtp = ctx.enter_context(tc.tile_pool(name="globT", bufs=2))
        mixp = ctx.enter_context(tc.tile_pool(name="mix", bufs=2))
        dtp = ctx.enter_context(tc.tile_pool(name="dt", bufs=3))
        l8p = ctx.enter_context(tc.tile_pool(name="l8", bufs=2))
        outp = ctx.enter_context(tc.tile_pool(name="ostage", bufs=3))
        gpool = ctx.enter_context(tc.tile_pool(name="gt", bufs=1))

        ps_st = ctx.enter_context(tc.tile_pool(name="ps_st", bufs=2, space="PSUM"))
        ps_pv = ctx.enter_context(tc.tile_pool(name="ps_pv", bufs=2, space="PSUM"))
        ps_sh = ctx.enter_context(tc.tile_pool(name="ps_sh", bufs=2, space="PSUM"))

        # ---- constants / persistent weights (outside the repeat loop) ----
        ident_f = const.tile([128, 128], F32)
        make_identity(nc, ident_f)
        ident_b = const.tile([128, 128], BF16)
        nc.vector.tensor_copy(out=ident_b, in_=ident_f)

        gw_sb = wpool.tile([128, 8, 2, D], FP8)
        nc.sync.dma_start(
            out=gw_sb, in_=gw_d.rearrange("p (c i n) -> p c i n", c=8, i=2)
        )
        ow_sb = wpool.tile([128, 8, D], BF16)
        nc.sync.dma_start(out=ow_sb, in_=ow_d.rearrange("(k p) n -> p k n", p=128))
        gb_sb = wpool.tile([128, 8], F32)
        nc.sync.dma_start(out=gb_sb, in_=gb_d.rearrange("m p -> p m"))
        obb = wpool.tile([128, D], F32)
        nc.sync.dma_start(out=obb, in_=ob_d.to_broadcast([128, D]))
        cbb = wpool.tile([G, D], F32)
        nc.sync.dma_start(out=cbb, in_=cb_d.to_broadcast([G, D]))

        def normalize_half(pv, out_half):
            """pv rows 0-63 = numerator, rows 64-127 = broadcast denominator.
            out_half <- pv[0:64] / pv[64:128]; the multiply is the PSUM
            evacuation (f32 psum in -> bf16 sbuf out). GPSIMD cannot touch
            PSUM on this target, so everything runs on DVE."""
            zl = rbp.tile([HD, WIN], F32, tag="zl")
            nc.scalar.activation(out=zl, in_=pv[HD:128, :], func=ACT_LN)
            rb = rbp.tile([HD, WIN], F32, tag="rb")
            nc.scalar.activation(out=rb, in_=zl, func=ACT_EXP, scale=-1.0)
            nc.vector.tensor_mul(out=out_half, in0=pv[0:HD, :], in1=rb)

        def emit_body():
            # ================= conv -> global tokens (as a closure) ========
            gt_tok = gpool.tile([128, 16, 128], BF16, tag="gt_tok")
            gtT = gpool.tile([128, 8, 128], BF16, tag="gtT")

            def emit_conv():
                xgp = gpool.tile([128, 4, 8, G], BF16, tag="xgp")
                nc.sync.dma_start(out=xgp, in_=xgp_d)

                nc.sync.dma_start(
                    out=gt_tok[G:128, :, :].rearrange("p a b -> p (a b)"),
                    in_=gma_d,
                )
                nc.vector.memset(gt_tok[0:G, :, HD:128], 1.0)

                comp = ps_sh.tile([128, WIN], F32, tag="sh")
                for kt in range(32):
                    cwc = cwp.tile([128, D], BF16, tag="cw")
                    nc.sync.dma_start(
                        out=cwc, in_=cw_d.rearrange("(k p) n -> k p n", p=128)[kt]
                    )
                    k, ib = kt // 8, kt % 8
                    nc.tensor.matmul(
                        comp[0:G, :], xgp[:, k, ib, :], cwc[:, 0:512],
                        start=(kt == 0), stop=(kt == 31), skip_group_check=True,
                    )
                    nc.tensor.matmul(
                        comp[G:128, :], xgp[:, k, ib, :], cwc[:, 512:1024],
                        start=(kt == 0), stop=(kt == 31), skip_group_check=True,
                    )
                for nt in range(2):
                    nc.vector.tensor_add(
                        out=gt_tok[0:G, nt * 8 : (nt + 1) * 8, 0:HD],
                        in0=comp[nt * G : (nt + 1) * G, :].rearrange(
                            "p (h d) -> p h d", h=8
                        ),
                        in1=cbb[:, nt * 512 : (nt + 1) * 512].rearrange(
                            "p (h d) -> p h d", h=8
                        ),
                    )

                nc.sync.dma_start(
                    out=gtT[:, :, G:128],
                    in_=gmt_d.rearrange("p (a b) -> p a b", b=G),
                )
                for hp in range(8):
                    ptf = ps_pv.tile([128, WIN], F32, tag="pv")
                    pt = ptf.bitcast(BF16)
                    for sub in range(2):
                        nc.tensor.transpose(
                            pt[0:G, sub * G : (sub + 1) * G],
                            gt_tok[0:G, 2 * hp + sub, 0:HD],
                            ident_b[0:G, 0:G],
                        )
                    for sub in range(2):
                        nc.vector.tensor_copy(
                            out=gtT[sub * HD : sub * HD + HD, hp, 0:G],
                            in_=pt[0:G, sub * G : (sub + 1) * G],
                        )

            # ========================= windows =========================
            # Window w's gate/comb/out units are emitted interleaved with
            # window w+1's attention blocks so the PE-only tail of each
            # window overlaps the ACT-throttled pairs phase of the next.
            def make_units(localT, globT, l8, tT, w):
                units = []

                def gate_unit(mt):
                    gp = ps_sh.tile([128, WIN], F32, tag="sh")
                    for c in range(8):
                        nc.tensor.matmul(
                            gp,
                            gw_sb[:, c, :, mt * 128 : (mt + 1) * 128],
                            l8[:, 2 * c : 2 * c + 2, :],
                            start=(c == 0),
                            stop=(c == 7),
                            perf_mode=mybir.MatmulPerfMode.DoubleRow,
                        )
                    s1 = rbp.tile([128, WIN], F32, tag="zl")
                    nc.scalar.activation(
                        out=s1, in_=gp, func=ACT_EXP, scale=-1.0,
                        bias=gb_sb[:, mt : mt + 1],
                    )
                    s2 = rbp.tile([128, WIN], F32, tag="rb")
                    nc.scalar.activation(out=s2, in_=s1, func=ACT_LN, bias=1.0)
                    nc.scalar.activation(
                        out=tT[:, mt, :], in_=s2, func=ACT_EXP, scale=-1.0
                    )

                def comb_unit():
                    # comb = glob + mix*(local-glob); in place in globT
                    for mt in range(8):
                        dt = dtp.tile([128, WIN], BF16, tag="dt")
                        nc.vector.tensor_sub(
                            out=dt, in0=localT[:, mt, :], in1=globT[:, mt, :]
                        )
                        nc.vector.tensor_mul(out=dt, in0=dt, in1=tT[:, mt, :])
                        nc.vector.tensor_add(
                            out=globT[:, mt, :], in0=globT[:, mt, :], in1=dt
                        )

                def out_unit(tt):
                    ost = outp.tile([128, D], F32, tag="ost")
                    for nt in range(2):
                        op = ps_sh.tile([128, WIN], F32, tag="sh")
                        for kt in range(8):
                            nc.tensor.matmul(
                                op,
                                globT[:, kt, tt * 128 : (tt + 1) * 128],
                                ow_sb[:, kt, nt * 512 : (nt + 1) * 512],
                                start=(kt == 0),
                                stop=(kt == 7),
                            )
                        nc.vector.tensor_add(
                            out=ost[:, nt * 512 : (nt + 1) * 512],
                            in0=op,
                            in1=obb[:, nt * 512 : (nt + 1) * 512],
                        )
                    nc.sync.dma_start(out=out_r[w * 4 + tt], in_=ost)

                for mt in range(8):
                    units.append(lambda mt=mt: gate_unit(mt))
                units.append(comb_unit)
                for tt in range(4):
                    units.append(lambda tt=tt: out_unit(tt))
                return units

            prev_units = []
            for w in range(NW):
                xw = xwp.tile([128, 4, 16, 128], BF16, tag="xw")
                for kt in range(4):
                    nc.sync.dma_start(
                        out=xw[:, kt, :, :].rearrange("p a b -> p (a b)"),
                        in_=xa_r[w * 4 + kt],
                    )

                # xT[p, hp, q] = xp[w*512+q, hp*128+p] (host-transposed)
                xT = xtp.tile([128, 8, WIN], BF16, tag="xT")
                nc.sync.dma_start(out=xT, in_=xt_d[:, :, w * WIN : (w + 1) * WIN])

                localT = ltp.tile([128, 8, WIN], BF16, tag="localT")
                globT = gtp.tile([128, 8, WIN], BF16, tag="globT")
                l8 = l8p.tile([128, 16, WIN], FP8, tag="l8")
                tT = mixp.tile([128, 8, WIN], BF16, tag="mx")

                def local_block(hp):
                    hA, hB = 2 * hp, 2 * hp + 1
                    xTA = xT[0:64, hp, :]      # head 2hp   [64, 512]
                    xTB = xT[64:128, hp, :]    # head 2hp+1 [64, 512]
                    pvA = ps_pv.tile([128, WIN], F32, tag="pv")
                    pvB = ps_pv.tile([128, WIN], F32, tag="pv")
                    for ktp in range(2):
                        sts = []
                        ets = []
                        for kk in range(2):
                            kt = 2 * ktp + kk
                            st = ps_st.tile([128, 2, WIN], F32, tag="st")
                            nc.tensor.matmul(
                                st[:, 0, :],
                                xTA[:, kt * 128 : (kt + 1) * 128], xTA,
                                start=True, stop=True,
                            )
                            nc.tensor.matmul(
                                st[:, 1, :],
                                xTB[:, kt * 128 : (kt + 1) * 128], xTB,
                                start=True, stop=True,
                            )
                            sts.append(st)
                        for kk in range(2):
                            et = expp.tile([128, 2, WIN], BF16, tag="et")
                            nc.scalar.activation(
                                out=et.rearrange("p a q -> p (a q)"),
                                in_=sts[kk].rearrange("p a q -> p (a q)"),
                                func=ACT_EXP, scale=SCALE,
                            )
                            ets.append(et)
                        for kk in range(2):
                            kt = 2 * ktp + kk
                            nc.tensor.matmul(
                                pvA, xw[:, kt, hA, :],
                                ets[kk][:, 0, :],
                                start=(kt == 0), stop=(kt == 3),
                            )
                            nc.tensor.matmul(
                                pvB, xw[:, kt, hB, :],
                                ets[kk][:, 1, :],
                                start=(kt == 0), stop=(kt == 3),
                            )
                    normalize_half(pvA, localT[0:64, hp, :])
                    normalize_half(pvB, localT[64:128, hp, :])
                    nc.gpsimd.tensor_copy(
                        out=l8[:, hp, :], in_=localT[:, hp, :]
                    )

                def global_block(hp):
                    hA, hB = 2 * hp, 2 * hp + 1
                    xTA = xT[0:64, hp, :]
                    xTB = xT[64:128, hp, :]
                    sg = ps_st.tile([128, 2, WIN], F32, tag="st")
                    nc.tensor.matmul(
                        sg[:, 0, :], gtT[0:64, hp, :], xTA, start=True, stop=True
                    )
                    nc.tensor.matmul(
                        sg[:, 1, :], gtT[64:128, hp, :], xTB, start=True, stop=True
                    )
                    eg = expp.tile([128, 2, WIN], BF16, tag="et")
                    nc.scalar.activation(
                        out=eg.rearrange("p a q -> p (a q)"),
                        in_=sg.rearrange("p a q -> p (a q)"),
                        func=ACT_EXP, scale=SCALE,
                    )
                    pgA = ps_pv.tile([128, WIN], F32, tag="pv")
                    nc.tensor.matmul(
                        pgA, gt_tok[:, hA, :],
                        eg[:, 0, :], start=True, stop=True,
                    )
                    pgB = ps_pv.tile([128, WIN], F32, tag="pv")
                    nc.tensor.matmul(
                        pgB, gt_tok[:, hB, :],
                        eg[:, 1, :], start=True, stop=True,
                    )
                    normalize_half(pgA, globT[0:64, hp, :])
                    normalize_half(pgB, globT[64:128, hp, :])
                    nc.gpsimd.tensor_copy(
                        out=l8[:, 8 + hp, :], in_=globT[:, hp, :]
                    )

                if w == 0:
                    # L-blocks first (no global-token dependency); the conv
                    # runs on the 48%-idle PE under ACT's exp backlog, then
                    # G-blocks. Units of the previous window: none (w=0).
                    for hp in range(8):
                        local_block(hp)
                    emit_conv()
                    for hp in range(8):
                        global_block(hp)
                else:
                    for hp in range(8):
                        local_block(hp)
                        global_block(hp)
                        for u in prev_units[2 * hp : 2 * hp + 2]:
                            u()

                prev_units = make_units(localT, globT, l8, tT, w)

            for u in prev_units:
                u()

        if nrep > 1 and os.environ.get("KERNEL_UNROLL"):
            for _ in range(nrep):
                emit_body()
        elif nrep > 1:
            with tc.For_i(0, nrep, 1):
                emit_body()
        else:
            emit_body()

    return nc


# ---------------------------------------------------------------------------
# host side: sharding, compile-once runner, gather
# ---------------------------------------------------------------------------


def shard_inputs(x, pos_emb, global_memory, conv_w, conv_b, gate_w, gate_b, out_w, out_b):
    x = np.asarray(x, np.float32)
    pos = np.asarray(pos_emb, np.float32)[:S]
    gmem = np.asarray(global_memory, np.float32)[0]          # [G, D]
    cw = np.ascontiguousarray(
        np.asarray(conv_w, np.float32).transpose(2, 1, 0).reshape(4 * D, D)
    ).astype(BF16_NP)
    cb = np.asarray(conv_b, np.float32)[None, :]
    gw = np.ascontiguousarray(
        np.asarray(gate_w, np.float32)
        .reshape(8, 2, 128, D)
        .transpose(2, 0, 1, 3)
        .reshape(128, 16 * D)
    ).astype(FP8_NP)
    gb = (-np.asarray(gate_b, np.float32)).reshape(8, 128)
    ow = np.asarray(out_w, np.float32).astype(BF16_NP)
    ob = np.asarray(out_b, np.float32)[None, :]

    # global-memory half of gt_tok: [G, 16, 128] bf16, [V_h | ones] per head
    gmem_b = gmem.astype(BF16_NP)
    gma = np.concatenate(
        [gmem_b.reshape(G, 16, HD), np.ones((G, 16, HD), BF16_NP)], axis=2
    ).reshape(G, 16 * 128)
    # global-memory feature-major for gtT: [128, 8, HD]
    gmt = np.ascontiguousarray(
        gmem_b.T.reshape(8, 2, HD, G).transpose(1, 2, 0, 3).reshape(128, 8 * G)
    )

    in_maps = []
    for c in range(N_CORES):
        b, off = c // 2, (c % 2) * SC
        xh = (x[b, off : off + SC] + pos[off : off + SC]).astype(BF16_NP)  # [SC, D]
        xa = np.concatenate(
            [xh.reshape(SC, 16, HD), np.ones((SC, 16, HD), BF16_NP)], axis=2
        ).reshape(SC, 2 * D)
        xt = np.ascontiguousarray(
            xh.T.reshape(8, 2, HD, SC).transpose(1, 2, 0, 3).reshape(128, 8, SC)
        )
        xg = (x[b, : 4 * G] + pos[: 4 * G]).astype(BF16_NP)                # [256, D]
        # xgp[p, k, ib, g] = xg[4g+k, ib*128+p]
        xgp = np.ascontiguousarray(
            xg.reshape(G, 4, 8, 128).transpose(3, 1, 2, 0).reshape(128, 4 * 8 * G)
        )
        in_maps.append(
            {
                "xa": xa,
                "xt": xt,
                "xgp": xgp,
                "gma": gma,
                "gmt": gmt,
                "cw": cw,
                "cb": cb,
                "gw": gw,
                "gb": gb,
                "ow": ow,
                "ob": ob,
            }
        )
    return in_maps


class Runner:
    """Compile a Bass program once; execute repeatedly on 8 cores via PJRT."""

    def __init__(self, nc, n_cores=N_CORES):
        bass2jax.install_neuronx_cc_hook()
        split_sync_waits(nc)  # walrus-compat: <=1 sem wait per instruction
        self.nc = nc
        self.n_cores = n_cores
        partition_name = nc.partition_id_tensor.name if nc.partition_id_tensor else None
        in_names, out_names, out_avals = [], [], []
        for alloc in nc.m.functions[0].allocations:
            if not isinstance(alloc, mybir.MemoryLocationSet):
                continue
            name = alloc.memorylocations[0].name
            if alloc.kind == "ExternalInput":
                if name != partition_name:
                    in_names.append(name)
            elif alloc.kind == "ExternalOutput":
                out_names.append(name)
                out_avals.append(
                    jax.core.ShapedArray(
                        tuple(alloc.tensor_shape), mybir.dt.np(alloc.dtype)
                    )
                )
        self.in_names, self.out_names, self.out_avals = in_names, out_names, out_avals
        n_params = len(in_names)
        all_in_names = list(in_names) + list(out_names)
        if partition_name is not None:
            all_in_names.append(partition_name)

        def _body(*args):
            operands = list(args)
            if partition_name is not None:
                operands.append(bass2jax.partition_id_tensor())
            outs = bass2jax._bass_exec_p.bind(
                *operands,
                out_avals=tuple(out_avals),
                in_names=tuple(all_in_names),
                out_names=tuple(out_names),
                lowering_input_output_aliases=(),
                sim_require_finite=False,
                sim_require_nnan=False,
                nc=nc,
            )
            return tuple(outs)

        devices = jax.devices()[:n_cores]
        self.mesh = Mesh(np.asarray(devices), ("core",))
        self.sharded = jax.jit(
            shard_map(
                _body,
                mesh=self.mesh,
                in_specs=(PartitionSpec("core"),) * (n_params + len(out_names)),
                out_specs=(PartitionSpec("core"),) * len(out_names),
                check_rep=False,
            )
        )

    def put(self, in_maps):
        concat = [
            np.concatenate(
                [np.asarray(in_maps[c][n]) for c in range(self.n_cores)], axis=0
            )
            for n in self.in_names
        ]
        for av in self.out_avals:
            concat.append(
                np.zeros((self.n_cores * av.shape[0], *av.shape[1:]), av.dtype)
            )
        return [jax.device_put(a) for a in concat]

    def run(self, dev_args):
        outs = self.sharded(*dev_args)
        jax.block_until_ready(outs)
        return outs

    def get(self, outs):
        res = []
        for c in range(self.n_cores):
            d = {}
            for i, n in enumerate(self.out_names):
                d[n] = np.asarray(outs[i]).reshape(
                    self.n_cores, *self.out_avals[i].shape
                )[c]
            res.append(d)
        return res


_RUNNERS: dict = {}


def get_runner(nrep: int = 1) -> Runner:
    if nrep not in _RUNNERS:
        _RUNNERS[nrep] = Runner(build_nc(nrep))
    return _RUNNERS[nrep]


def kernel(**inputs) -> np.ndarray:
    in_maps = shard_inputs(**inputs)
    runner = get_runner(1)
    res = runner.get(runner.run(runner.put(in_maps)))
    out = np.empty((B, S, D), np.float32)
    for c in range(N_CORES):
        b, off = c // 2, (c % 2) * SC
        out[b, off : off + SC] = res[c]["out"]
    return out
